# revision 20
# baseline (speedup 1.0000x reference)
"""3-level Haar DWT feature kernel for Trainium2 (8 NeuronCores, data-parallel).

Full input x: [256, 131072] f32. Output: [256, 131072] f32 =
concat([cA3, cD3, cD2, cD1], axis=1) per row (pywt wavedec order).

Sharding: batch dim split 8 ways (32 rows per core), no cross-core comm.

Layout: a group of R=8 rows is one [128, F=8192] SBUF tile where partition
p = r_local*16 + p_sub holds row elements [p_sub*F, (p_sub+1)*F).  Haar
pairs (2k, 2k+1) are adjacent along the free dim within one partition at
every level, so each level is two stride-2 tensor_tensor butterflies on
the DVE.  Segment k of a partition's input slice lands at
out[r, seg_base + p_sub*(seg_len/16) + f], so each output segment is its
own rectangular (r, p, f) store AP; per group that is 1 load (SP HWDGE
ring) + 4 segment stores (ACT ring), the two rings streaming
concurrently.  Fewer/bigger DMAs win on this part: R=2 (80 DMAs) measured
~3x slower than R=8 (20 DMAs); gpsimd butterfly offload also measured
slower than leaving everything on the DVE.

Host pre-scales x by 1/sqrt(2), so level 1 needs no output scaling at all
(cD1 = e-o, cA1 = e+o exactly, and the DVE writes cD1 straight to its
bf16 store tile); the remaining scales (c, 1/2, 1/2) ride on the
otherwise-idle ScalarEngine as copy-with-scale casts into the bf16 store
tiles.  Intermediate sums stay f32.

Precision: harness gate is rel_err < 2e-2.  This kernel is purely
memory-bound (copy roofline of the same f32 byte volume measures ~93us),
so bf16 I/O — inputs quantized host-side, outputs stored bf16 and widened
host-side — halves HBM traffic and is the single biggest win; it costs
~2.4e-3 rel err (vs 2e-2 gate).  fp8-e4m3 input was evaluated and fails
the gate (2.65e-2 quantization error alone).
"""

import numpy as np
import ml_dtypes

import concourse.bacc as bacc
import concourse.bass as bass
import concourse.mybir as mybir
from concourse.tile import TileContext
from concourse.bass_utils import run_bass_kernel_spmd

INV_SQRT2 = 0.7071067811865476
C1 = INV_SQRT2

N_CORES = 8
B, L = 256, 131072
ROWS = B // N_CORES     # 32 rows per core

FP32 = mybir.dt.float32
BF16 = mybir.dt.bfloat16
NP_BF16 = np.dtype(ml_dtypes.bfloat16)

SUB = mybir.AluOpType.subtract
ADD = mybir.AluOpType.add


def _pairs(ap):
    """[128, N] AP -> (even, odd) stride-2 APs of shape [128, N//2]."""
    p3 = ap.rearrange("p (n two) -> p n two", two=2)
    return p3[:, :, 0], p3[:, :, 1]


def _group_ap(t, g, rows_per_group):
    p_sub = 128 // rows_per_group
    rows = slice(g * rows_per_group, (g + 1) * rows_per_group)
    return t[rows].rearrange("r (p f) -> (r p) f", p=p_sub)


# ---------------------------------------------------------------- v8 (orig)
def _emit_v8(nc, tc, x, out):
    R = 8
    P_SUB = 16
    F = 8192
    C2, C3 = 0.5, 0.5 * INV_SQRT2
    with (
        tc.tile_pool(name="xin", bufs=2) as xin_pool,
        tc.tile_pool(name="mid", bufs=1) as mid_pool,
        tc.tile_pool(name="outs", bufs=2) as out_pool,
    ):
        for g in range(4):
            rows = slice(g * R, (g + 1) * R)
            xt = xin_pool.tile([128, F], FP32, tag="xt")
            nc.sync.dma_start(out=xt[:], in_=_group_ap(x, g, R))

            def store(tile, seg_lo, seg_hi):
                nc.scalar.dma_start(
                    out=out[rows, seg_lo:seg_hi].rearrange(
                        "r (p f) -> r p f", p=P_SUB),
                    in_=tile[:],
                )

            def level(src_ap, n_out, tag, cd_scale, cd_seg,
                      ca_scale=None, ca_seg=None):
                ev, od = _pairs(src_ap)
                du = mid_pool.tile([128, n_out], FP32, tag=f"du{tag}")
                nc.vector.tensor_tensor(out=du[:], in0=ev, in1=od, op=SUB)
                d = out_pool.tile([128, n_out], FP32, tag=f"d{tag}")
                nc.scalar.mul(d[:], du[:], cd_scale)
                store(d, *cd_seg)

                if ca_seg is None:
                    s = mid_pool.tile([128, n_out], FP32, tag=f"s{tag}")
                    nc.vector.tensor_tensor(out=s[:], in0=ev, in1=od, op=ADD)
                    return s
                su = mid_pool.tile([128, n_out], FP32, tag=f"su{tag}")
                nc.vector.tensor_tensor(out=su[:], in0=ev, in1=od, op=ADD)
                a = out_pool.tile([128, n_out], FP32, tag="a3")
                nc.scalar.mul(a[:], su[:], ca_scale)
                store(a, *ca_seg)
                return a

            s1 = level(xt[:], F // 2, "1", C1, (L // 2, L))
            s2 = level(s1[:], F // 4, "2", 0.5, (L // 4, L // 2))
            level(s2[:], F // 8, "3", C3, (L // 8, L // 4),
                  ca_scale=C3, ca_seg=(0, L // 8))


# ------------------------------------------------------- v9copy (DMA probe)
def _emit_v9copy(nc, tc, x, out, dt=FP32):
    R, F = 8, 8192
    with tc.tile_pool(name="xin", bufs=2) as xin_pool:
        for g in range(ROWS // R):
            xt = xin_pool.tile([128, F], dt, tag="xt")
            nc.sync.dma_start(out=xt[:], in_=_group_ap(x, g, R))
            nc.scalar.dma_start(out=_group_ap(out, g, R), in_=xt[:])


# ------------------------------------------- prescaled per-segment emitter
def _emit_dwt(nc, tc, x, out, in_dt, out_dt, rows_per_group=4, bufs=3,
              mid_dt=FP32, pool_levels=(), mid_bufs=2, a3_on_sp=None):
    """Host pre-scaled by 1/sqrt2: level-1 outputs need no scaling.

    Per group: 1 load (SP ring) and 3 segment stores: cD1 and cD2 on the
    ACT ring, the combined [cA3|cD3] tile on the SP ring (balances ring
    bytes: SP = loads + 1/4 of stores).  Segment k of a partition's input
    slice lands at out[r, seg_base + p*(seg_len/P_SUB) + f], so each
    segment store is its own rectangular (r, p, f) AP; cA3/cD3 share one
    (r, p, seg, f) AP since they have equal length.
    """
    R = rows_per_group
    P_SUB = 128 // R
    F = (L * R) // 128
    n_groups = ROWS // R

    def seg_ap(g, lo, hi):
        rows = slice(g * R, (g + 1) * R)
        return out[rows, lo:hi].rearrange("r (p f) -> r p f", p=P_SUB)

    with (
        tc.tile_pool(name="xin", bufs=bufs) as xin_pool,
        tc.tile_pool(name="mid", bufs=mid_bufs) as mid_pool,
        tc.tile_pool(name="outs", bufs=bufs) as out_pool,
    ):
        eng = {lvl: (nc.gpsimd if lvl in pool_levels else nc.vector)
               for lvl in (1, 2, 3)}
        for g in range(n_groups):
            xt = xin_pool.tile([128, F], in_dt, tag="xt")
            nc.sync.dma_start(out=xt[:], in_=_group_ap(x, g, R))

            # level 1: cD1/cA1 exact (input pre-scaled by c)
            ev, od = _pairs(xt[:])
            d1 = out_pool.tile([128, F // 2], out_dt, tag="d1")
            eng[1].tensor_tensor(out=d1[:], in0=ev, in1=od, op=SUB)
            nc.scalar.dma_start(out=seg_ap(g, L // 2, L), in_=d1[:])
            s1 = mid_pool.tile([128, F // 2], mid_dt, tag="s1")
            eng[1].tensor_tensor(out=s1[:], in0=ev, in1=od, op=ADD)

            # level 2: cD2 = c*(e-o), carry s2 = (e+o) = cA2/c
            ev, od = _pairs(s1[:])
            du2 = mid_pool.tile([128, F // 4], mid_dt, tag="du2")
            eng[2].tensor_tensor(out=du2[:], in0=ev, in1=od, op=SUB)
            d2 = out_pool.tile([128, F // 4], out_dt, tag="d2")
            nc.scalar.mul(d2[:], du2[:], C1)
            nc.scalar.dma_start(out=seg_ap(g, L // 4, L // 2), in_=d2[:])
            s2 = mid_pool.tile([128, F // 4], mid_dt, tag="s2")
            eng[2].tensor_tensor(out=s2[:], in0=ev, in1=od, op=ADD)

            # level 3: cD3 = c^2*(e-o), cA3 = c^2*(e+o), c^2 = 1/2
            ev, od = _pairs(s2[:])
            du3 = mid_pool.tile([128, F // 8], mid_dt, tag="du3")
            eng[3].tensor_tensor(out=du3[:], in0=ev, in1=od, op=SUB)
            su3 = mid_pool.tile([128, F // 8], mid_dt, tag="su3")
            eng[3].tensor_tensor(out=su3[:], in0=ev, in1=od, op=ADD)
            d3 = out_pool.tile([128, F // 8], out_dt, tag="d3")
            nc.scalar.mul(d3[:], du3[:], 0.5)
            nc.scalar.dma_start(out=seg_ap(g, L // 8, L // 4), in_=d3[:])
            a3 = out_pool.tile([128, F // 8], out_dt, tag="a3")
            nc.scalar.mul(a3[:], su3[:], 0.5)
            # small cA3 store can ride the SP ring — but a not-yet-ready
            # store queued on SP head-of-line-blocks the next group's load,
            # so default is the ACT ring with all other stores
            use_sp = in_dt == BF16 if a3_on_sp is None else a3_on_sp
            seng = nc.sync if use_sp else nc.scalar
            seng.dma_start(out=seg_ap(g, 0, L // 8), in_=a3[:])


# version name -> (emitter, in dtype, out dtype, host prescale)
VERSIONS = {
    "v8":     (_emit_v8, FP32, FP32, False),
    "v9copy": (_emit_v9copy, FP32, FP32, False),
    "v16":    (lambda nc, tc, x, o: _emit_dwt(nc, tc, x, o, FP32, BF16,
               rows_per_group=4, bufs=3), FP32, BF16, True),
    "v17":    (lambda nc, tc, x, o: _emit_dwt(nc, tc, x, o, BF16, BF16,
               rows_per_group=4, bufs=3), BF16, BF16, True),
    "v17r8":  (lambda nc, tc, x, o: _emit_dwt(nc, tc, x, o, BF16, BF16,
               rows_per_group=8, bufs=2), BF16, BF16, True),
    "v17r8b": (lambda nc, tc, x, o: _emit_dwt(nc, tc, x, o, BF16, BF16,
               rows_per_group=8, bufs=3), BF16, BF16, True),
    "v9cbf":  (lambda nc, tc, x, o: _emit_v9copy(nc, tc, x, o, BF16),
               BF16, BF16, False),
    "v17r8p": (lambda nc, tc, x, o: _emit_dwt(nc, tc, x, o, BF16, BF16,
               rows_per_group=8, bufs=2, pool_levels=(2,)),
               BF16, BF16, True),
    "v17r16": (lambda nc, tc, x, o: _emit_dwt(nc, tc, x, o, BF16, BF16,
               rows_per_group=16, bufs=2, mid_dt=BF16, mid_bufs=1),
               BF16, BF16, True),
    "v17r8a": (lambda nc, tc, x, o: _emit_dwt(nc, tc, x, o, BF16, BF16,
               rows_per_group=8, bufs=2, a3_on_sp=False),
               BF16, BF16, True),
    "v17r8pa": (lambda nc, tc, x, o: _emit_dwt(nc, tc, x, o, BF16, BF16,
                rows_per_group=8, bufs=2, pool_levels=(2,), a3_on_sp=False),
                BF16, BF16, True),
    "v17r2":  (lambda nc, tc, x, o: _emit_dwt(nc, tc, x, o, BF16, BF16,
               rows_per_group=2, bufs=4), BF16, BF16, True),
    "v18":    (lambda nc, tc, x, o: _emit_dwt(nc, tc, x, o, FP32, FP32,
               rows_per_group=4, bufs=3), FP32, FP32, True),
}

KERNEL_VERSION = "v17r8a"


def np_dt(dt):
    return {FP32: np.dtype(np.float32), BF16: NP_BF16}[dt]


def prep_input(x, version=None):
    """Full [B, L] f32 -> np array ready for device upload (sharded later)."""
    version = version or KERNEL_VERSION
    _, in_dt, _, prescale = VERSIONS[version]
    x = np.asarray(x, dtype=np.float32)
    if prescale:
        x = x * np.float32(INV_SQRT2)
    return np.ascontiguousarray(x.astype(np_dt(in_dt)))


def build_nc(version=None, reps=1):
    version = version or KERNEL_VERSION
    emit, in_dt, out_dt, _ = VERSIONS[version]
    nc = bacc.Bacc(
        "TRN2",
        target_bir_lowering=False,
        debug=False,
        num_devices=N_CORES,
    )
    x = nc.dram_tensor("x", [ROWS, L], in_dt, kind="ExternalInput")
    out = nc.dram_tensor("out", [ROWS, L], out_dt, kind="ExternalOutput")
    with TileContext(nc) as tc:
        for _ in range(reps):
            emit(nc, tc, x, out)
    nc.compile()
    return nc


_NC_CACHE = {}


def _get_nc(version):
    if version not in _NC_CACHE:
        _NC_CACHE[version] = build_nc(version)
    return _NC_CACHE[version]


def run_sharded(x, version=None, **kwargs):
    """Run on 8 cores; returns (full_output_f32, BassKernelResults)."""
    version = version or KERNEL_VERSION
    x = np.asarray(x)
    assert x.shape == (B, L), x.shape
    xdev = prep_input(x, version)
    nc = _get_nc(version)
    in_maps = [
        {"x": np.ascontiguousarray(xdev[i * ROWS:(i + 1) * ROWS])}
        for i in range(N_CORES)
    ]
    res = run_bass_kernel_spmd(nc, in_maps, list(range(N_CORES)), **kwargs)
    full = np.concatenate(
        [res.results[i]["out"] for i in range(N_CORES)], axis=0)
    return np.asarray(full).astype(np.float32), res


def kernel(x):
    out, _ = run_sharded(x)
    return out
